# revision 43
# baseline (speedup 1.0000x reference)
"""Trainium2 Bass kernel for nn_NodeRNN (masked single-step LSTM over N nodes).

Strategy: the reference computes the LSTM step everywhere and then keeps the
old state for inactive nodes (ts_mask != 1). Equivalently: gather the active
rows, run the LSTM step on just those, scatter back. The gather/scatter is
pure data routing, done host-side during staging (where the baseline already
transposes); only active nodes ever touch the device. That halves HBM traffic
and every engine's work.

All per-node data is staged FEATURE-MAJOR and in bf16 (tolerance is 2e-2;
bf16 keeps us ~3 orders of magnitude under it), packed into ONE interleaved
dram stream per core laid out [128, NT, 6, T] so each tile is a single DMA of
128 x 12KB contiguous descriptors:
    chunk 0..1: hvv.T      chunk 2..3: Hv.T      chunk 4: hv.T   chunk 5: cv.T

Per 1024-node tile (two 512-node matmul subtiles per PSUM bank):
  x_ps  = [W_pos @ xv.T + b_pos ; W_hid @ X.T + b_hid]   (PE; biases folded
          into the matmul via a ones-row in the aux stream)
  x     = relu(x_ps)                                     (DVE max, -> bf16)
  gates = W_ih @ x + W_hh @ hv.T (+ fused bias via ACT)  (PE)
  i,f,o = sigmoid, g = tanh                              (ACT, -> bf16)
  c_new = f*cv + i*g ; h_new = o*tanh(c_new)             (DVE + one ACT tanh)
Outputs leave as bf16 [128, NT, 2, T]; host scatters them back into f32
copies of hv_tm1/cv_tm1 (inactive rows therefore stay bit-exact).
Emission is software-pipelined (stage A of tile t+1 before stage B of tile t).
"""
import sys

sys.path.insert(0, "/opt/trn_rl_repo")

import numpy as np
import ml_dtypes

import concourse.bacc as bacc
import concourse.tile as tile
from concourse import mybir
from concourse.bass_utils import run_bass_kernel_spmd

f32 = mybir.dt.float32
bf16 = mybir.dt.bfloat16
fp8 = mybir.dt.float8e4
AF = mybir.ActivationFunctionType
ALU = mybir.AluOpType
BF16 = ml_dtypes.bfloat16

N = 262144
NCORES = 8
T = 1024                  # nodes per tile (DMA + elementwise granularity)
TS = 512                  # matmul subtile (PSUM bank = 512 f32)
EMBED = 64
EDGE_H = 256
NODE_H = 128

# weight block layout: [128, CWF] bf16, free-dim offsets
CO_WHID = 0               # 4 chunks x 128 cols; cols 64:128 of chunk c = W_hid.T chunk
CO_WIH = 512              # W_ih.T [128, 512]
CO_WHH = 1024             # W_hh.T [128, 512]
CO_WP = 1536              # [3, 128]: rows 0:2 = W_pos.T | 0, row 2 = concat(b_pos, b_hid)
CWF = 1664

GATE_FUNCS = [AF.Sigmoid, AF.Sigmoid, AF.Tanh, AF.Sigmoid]  # i, f, g, o

_cached = {}


def build_nc(nt, tw):
    """nt full tiles of T nodes plus one tail tile of tw (0 or TS) nodes."""
    ns = nt * T + tw
    nc = bacc.Bacc(target_bir_lowering=False)
    din_d = nc.dram_tensor("din", [128, ns * 6], bf16, kind="ExternalInput")
    aux_d = nc.dram_tensor("aux", [3, ns], bf16, kind="ExternalInput")
    cw_d = nc.dram_tensor("cw", [128, CWF], bf16, kind="ExternalInput")
    cb_d = nc.dram_tensor("cb", [128, 4], f32, kind="ExternalInput")
    # h is bounded by |o*tanh| <= 1, so fp8e4m3 (abs err <= 0.031 vs the
    # 0.108 tolerance budget) is safe; c keeps bf16. The SWDGE out-DMA does
    # the bf16 -> fp8 cast in flight.
    h_d = nc.dram_tensor("h_out", [128, ns], fp8, kind="ExternalOutput")
    c_d = nc.dram_tensor("c_out", [128, ns], bf16, kind="ExternalOutput")

    def tile_view(dram, t, c):
        w = T if t < nt else tw
        off = t * c * T
        return dram[:, off:off + c * w].rearrange("p (c n) -> p c n", c=c)

    with tile.TileContext(nc) as tc:
        with (
            tc.tile_pool(name="const", bufs=1) as cpool,
            tc.tile_pool(name="din", bufs=6) as dinp,
            tc.tile_pool(name="xsb", bufs=3) as xsbp,
            tc.tile_pool(name="gact", bufs=3) as gactp,
            tc.tile_pool(name="tmp", bufs=3) as tmpp,
            tc.tile_pool(name="hcn", bufs=3) as hcnp,
            tc.tile_pool(name="ps_x", bufs=1, space="PSUM") as psx,
            tc.tile_pool(name="ps_g", bufs=3, space="PSUM") as psg,
        ):
            cw = cpool.tile([128, CWF], bf16)
            nc.sync.dma_start(cw[:], cw_d[:])
            cb = cpool.tile([128, 4], f32)
            nc.sync.dma_start(cb[:], cb_d[:])
            # whole-run xv/ones stream: one small DMA (dispatched after the
            # first tile's din so the first matmuls aren't queued behind it)
            aux_sb = cpool.tile([3, ns], bf16)

            # warmup stream: absorbs the cw DMA wait and accumulates the ~3us
            # of continuous PE activity that ramps the HAM clock to full speed
            # before the first real matmul
            warm = psx.tile([128, T], f32, tag="x")
            for _ in range(16):
                nc.tensor.matmul(warm[:, 0:256], cw[0:3, CO_WP:CO_WP + 128],
                                 cw[0:3, 0:256], start=True, stop=True)

            stash = {}

            def stage_a(t, first=False):
                w = T if t < nt else tw
                src = tile_view(din_d, t, 6)
                din_t = dinp.tile([128, 6, T], bf16, tag="din")
                nc.sync.dma_start(din_t[:, :, 0:w], src[:])
                if first:
                    nc.sync.dma_start(aux_sb[:], aux_d[:])

                # x_ps: partitions 0:64 e_v, 64:128 a_v, biases included via
                # the aux ones-row
                x_ps = psx.tile([128, T], f32, tag="x")
                for k0 in range(0, w, TS):
                    ksl = slice(k0, min(k0 + TS, w))
                    for c in range(4):
                        nc.tensor.matmul(
                            x_ps[:, ksl],
                            cw[:, CO_WHID + 128 * c:CO_WHID + 128 * (c + 1)],
                            din_t[:, c, ksl], start=(c == 0), stop=False)
                    nc.tensor.matmul(
                        x_ps[:, ksl], cw[0:3, CO_WP:CO_WP + 128],
                        aux_sb[0:3, t * T + ksl.start:t * T + ksl.stop],
                        start=False, stop=True)

                # x = relu(x_ps) on the DVE, rounded to bf16 for the gate matmuls
                x_sb = xsbp.tile([128, T], bf16, tag="x_sb")
                nc.vector.tensor_scalar_max(x_sb[:, 0:w], x_ps[:, 0:w], 0.0)
                stash[t] = (din_t, x_sb)

            def stage_b(t, nsplit=1, final=False):
                din_t, x_sb = stash.pop(t)
                hcn = hcnp.tile([128, 2, T], bf16, tag="hcn")
                w = (T if t < nt else tw) // nsplit
                for h in range(nsplit):
                    hsl = slice(h * w, (h + 1) * w)
                    gact = [None] * 4
                    # per gate chunk j: g_ps_j = W_hh.T_j @ hv.T + W_ih.T_j @ x.
                    # f first so t1 = f*cv can start early on the DVE.
                    for j in (1, 0, 2, 3):
                        gp = psg.tile([128, T], f32, tag="g")
                        for k0 in range(h * w, (h + 1) * w, TS):
                            ksl = slice(k0, min(k0 + TS, (h + 1) * w))
                            nc.tensor.matmul(
                                gp[:, ksl],
                                cw[:, CO_WHH + 128 * j:CO_WHH + 128 * (j + 1)],
                                din_t[:, 4, ksl], start=True, stop=False)
                            nc.tensor.matmul(
                                gp[:, ksl],
                                cw[:, CO_WIH + 128 * j:CO_WIH + 128 * (j + 1)],
                                x_sb[:, ksl], start=False, stop=True)
                        ga = gactp.tile([128, T], bf16, tag=f"g{j}")
                        gact[j] = ga
                        nc.scalar.activation(ga[:, hsl], gp[:, hsl], GATE_FUNCS[j],
                                             bias=cb[:, j:j + 1])
                        if j == 1:
                            # t1 = f * cv while the other gates are in flight
                            t1 = tmpp.tile([128, T], bf16, tag="t1")
                            nc.vector.tensor_tensor(t1[:, hsl], ga[:, hsl],
                                                    din_t[:, 5, hsl], ALU.mult)
                    i_s, f_s, g_t, o_s = gact

                    t2 = tmpp.tile([128, T], bf16, tag="t2")
                    th = tmpp.tile([128, T], bf16, tag="th")
                    nc.vector.tensor_tensor(t2[:, hsl], i_s[:, hsl], g_t[:, hsl],
                                            ALU.mult)
                    nc.vector.tensor_tensor(hcn[:, 1, hsl], t1[:, hsl], t2[:, hsl],
                                            ALU.add)
                    nc.scalar.activation(th[:, hsl], hcn[:, 1, hsl], AF.Tanh)
                    nc.vector.tensor_tensor(hcn[:, 0, hsl], o_s[:, hsl], th[:, hsl],
                                            ALU.mult)
                    # outs go via the GPSIMD software DGE so the sync HW queue
                    # carries only the din stream; h additionally needs SWDGE
                    # for the bf16->fp8 cast. The final tile's c goes via sync
                    # (no din left there) to skip the slow SWDGE drain.
                    nsl = slice(t * T + hsl.start, t * T + hsl.stop)
                    nc.gpsimd.dma_start(h_d[:, nsl], hcn[:, 0, hsl])
                    eng = nc.sync if final else nc.gpsimd
                    eng.dma_start(c_d[:, nsl], hcn[:, 1, hsl])

            # tail tile first: its half-size din lands soonest, so the PE
            # starts real work earlier; full tiles stream behind it
            order = ([nt] if tw else []) + list(range(nt))
            for i in range(len(order) + 1):
                if i < len(order):
                    stage_a(order[i], first=(i == 0))
                if i >= 1:
                    t = order[i - 1]
                    last = i - 1 == len(order) - 1
                    stage_b(t, nsplit=2 if last and t < nt else 1, final=last)

    nc.finalize()
    return nc


def _stage_weights(W_pos, b_pos, W_hid, b_hid, W_ih, b_ih, W_hh, b_hh):
    cw = np.zeros((128, CWF), dtype=np.float32)
    whid_t = np.ascontiguousarray(W_hid.T)          # [512, 64]
    for c in range(4):
        cw[:, CO_WHID + 128 * c + 64:CO_WHID + 128 * (c + 1)] = whid_t[128 * c:128 * (c + 1)]
    cw[:, CO_WIH:CO_WIH + 512] = W_ih.T             # [128, 512]
    cw[:, CO_WHH:CO_WHH + 512] = W_hh.T
    cw[0:2, CO_WP:CO_WP + 64] = W_pos.T             # [2, 64]
    cw[2, CO_WP:CO_WP + 64] = b_pos
    cw[2, CO_WP + 64:CO_WP + 128] = b_hid
    cb = np.zeros((128, 4), dtype=np.float32)
    cb[:, :] = (b_ih + b_hh).reshape(4, 128).T
    return cw.astype(BF16), cb


def _stage_inputs(Hv_t, hvv_t, xv_t, hv_tm1, cv_tm1, act, ncap, weights):
    """Gather active rows, pad to ncap total, stage feature-major bf16."""
    cw, cb = _stage_weights(**weights)
    actp = np.pad(act, (0, ncap - act.size), mode="edge") if act.size < ncap else act
    ns = ncap // NCORES
    nt, tw = ns // T, ns % T
    in_maps = []
    for s in range(NCORES):
        idx = actp[s * ns:(s + 1) * ns]
        buf = np.empty((ns, 768), dtype=np.float32)
        buf[:, 0:256] = hvv_t[idx]
        buf[:, 256:512] = Hv_t[idx]
        buf[:, 512:640] = hv_tm1[idx]
        buf[:, 640:768] = cv_tm1[idx]
        # per tile: din[p, c, n] = buf_tile[n, c*128+p]
        bufb = buf.astype(BF16)
        din = np.empty((128, ns * 6), dtype=BF16)
        din[:, 0:nt * 6 * T] = (bufb[0:nt * T].reshape(nt, T, 6, 128)
                                .transpose(3, 0, 2, 1).reshape(128, nt * 6 * T))
        if tw:
            din[:, nt * 6 * T:] = (bufb[nt * T:].reshape(tw, 6, 128)
                                   .transpose(2, 1, 0).reshape(128, 6 * tw))
        aux = np.empty((3, ns), dtype=np.float32)
        aux[0:2] = xv_t[idx].T
        aux[2] = 1.0
        in_maps.append(dict(din=din, aux=aux.astype(BF16), cw=cw, cb=cb))
    return in_maps, actp


def run(inputs, trace=False):
    """Stage, run on 8 cores, unstage. Returns ((hv_t, cv_t), BassKernelResults)."""
    inputs = {k: np.asarray(v) for k, v in inputs.items()}
    weights = {k: inputs[k] for k in ["W_pos", "b_pos", "W_hid", "b_hid",
                                      "W_ih", "b_ih", "W_hh", "b_hh"]}
    act = np.flatnonzero(inputs["ts_mask"][:, 0] == 1)
    if act.size == 0:
        return (inputs["hv_tm1"].copy(), inputs["cv_tm1"].copy()), None
    grain = NCORES * 128
    ncap = -(-act.size // grain) * grain
    ns = ncap // NCORES
    nt, tw = ns // T, ns % T

    in_maps, actp = _stage_inputs(inputs["Hv_t"], inputs["hvv_t"], inputs["xv_t"],
                                  inputs["hv_tm1"], inputs["cv_tm1"], act, ncap,
                                  weights)
    if (nt, tw) not in _cached:
        _cached[(nt, tw)] = build_nc(nt, tw)
    res = run_bass_kernel_spmd(_cached[(nt, tw)], in_maps,
                               core_ids=list(range(NCORES)), trace=trace)
    hv_out = inputs["hv_tm1"].astype(np.float32, copy=True)
    cv_out = inputs["cv_tm1"].astype(np.float32, copy=True)
    na = act.size
    for s in range(NCORES):
        lo, hi = s * ns, (s + 1) * ns
        if lo >= na:
            break
        n_keep = min(hi, na) - lo
        h = res.results[s]["h_out"].astype(np.float32)
        c = res.results[s]["c_out"].astype(np.float32)
        hv_out[act[lo:lo + n_keep]] = h.T[:n_keep]
        cv_out[act[lo:lo + n_keep]] = c.T[:n_keep]
    return (hv_out, cv_out), res


def kernel(**inputs):
    out, _ = run(inputs, trace=False)
    return out


# revision 47
# speedup vs baseline: 1.1797x; 1.1797x over previous
"""Trainium2 Bass kernel for nn_NodeRNN (masked single-step LSTM over N nodes).

Strategy: the reference computes the LSTM step everywhere and then keeps the
old state for inactive nodes (ts_mask != 1). Equivalently: gather the active
rows, run the LSTM step on just those, scatter back. The gather/scatter is
pure data routing, done host-side during staging (where the baseline already
transposes); only active nodes ever touch the device. That halves HBM traffic
and every engine's work.

All per-node data is staged FEATURE-MAJOR and in bf16 (tolerance is 2e-2;
bf16 keeps us ~3 orders of magnitude under it), packed into ONE interleaved
dram stream per core laid out [128, NT, 6, T] so each tile is a single DMA of
128 x 12KB contiguous descriptors:
    chunk 0..1: hvv.T      chunk 2..3: Hv.T      chunk 4: hv.T   chunk 5: cv.T

Per 1024-node tile (two 512-node matmul subtiles per PSUM bank):
  x_ps  = [W_pos @ xv.T + b_pos ; W_hid @ X.T + b_hid]   (PE; biases folded
          into the matmul via a ones-row in the aux stream)
  x     = relu(x_ps)                                     (DVE max, -> bf16)
  gates = W_ih @ x + W_hh @ hv.T (+ fused bias via ACT)  (PE)
  i,f,o = sigmoid, g = tanh                              (ACT, -> bf16)
  c_new = f*cv + i*g ; h_new = o*tanh(c_new)             (DVE + one ACT tanh)
Outputs leave as bf16 [128, NT, 2, T]; host scatters them back into f32
copies of hv_tm1/cv_tm1 (inactive rows therefore stay bit-exact).
Emission is software-pipelined (stage A of tile t+1 before stage B of tile t).
"""
import sys

sys.path.insert(0, "/opt/trn_rl_repo")

import numpy as np
import ml_dtypes

import concourse.bacc as bacc
import concourse.tile as tile
from concourse import mybir
from concourse.bass_utils import run_bass_kernel_spmd

f32 = mybir.dt.float32
bf16 = mybir.dt.bfloat16
fp8 = mybir.dt.float8e4
AF = mybir.ActivationFunctionType
ALU = mybir.AluOpType
BF16 = ml_dtypes.bfloat16

N = 262144
NCORES = 8
T = 1024                  # nodes per tile (DMA + elementwise granularity)
TS = 512                  # matmul subtile (PSUM bank = 512 f32)
EMBED = 64
EDGE_H = 256
NODE_H = 128

# weight block layout: [128, CWF] bf16, free-dim offsets
CO_WHID = 0               # 4 chunks x 128 cols; cols 64:128 of chunk c = W_hid.T chunk
CO_WIH = 512              # W_ih.T [128, 512]
CO_WHH = 1024             # W_hh.T [128, 512]
CO_WP = 1536              # [3, 128]: rows 0:2 = W_pos.T | 0, row 2 = concat(b_pos, b_hid)
CWF = 1664

GATE_FUNCS = [AF.Sigmoid, AF.Sigmoid, AF.Tanh, AF.Sigmoid]  # i, f, g, o

_cached = {}


def build_nc(nt, tw):
    """nt full tiles of T nodes plus one tail tile of tw (0 or TS) nodes."""
    ns = nt * T + tw
    nc = bacc.Bacc(target_bir_lowering=False)
    din_d = nc.dram_tensor("din", [128, ns * 6], bf16, kind="ExternalInput")
    aux_d = nc.dram_tensor("aux", [3, ns], bf16, kind="ExternalInput")
    cw_d = nc.dram_tensor("cw", [128, CWF], bf16, kind="ExternalInput")
    cb_d = nc.dram_tensor("cb", [128, 4], f32, kind="ExternalInput")
    out_d = nc.dram_tensor("hc_out", [128, ns * 2], bf16, kind="ExternalOutput")

    def tile_view(dram, t, c):
        w = T if t < nt else tw
        off = t * c * T
        return dram[:, off:off + c * w].rearrange("p (c n) -> p c n", c=c)

    with tile.TileContext(nc) as tc:
        with (
            tc.tile_pool(name="const", bufs=1) as cpool,
            tc.tile_pool(name="din", bufs=6) as dinp,
            tc.tile_pool(name="xsb", bufs=3) as xsbp,
            tc.tile_pool(name="gact", bufs=3) as gactp,
            tc.tile_pool(name="tmp", bufs=3) as tmpp,
            tc.tile_pool(name="hcn", bufs=3) as hcnp,
            tc.tile_pool(name="ps_x", bufs=1, space="PSUM") as psx,
            tc.tile_pool(name="ps_g", bufs=3, space="PSUM") as psg,
        ):
            cw = cpool.tile([128, CWF], bf16)
            nc.sync.dma_start(cw[:], cw_d[:])
            cb = cpool.tile([128, 4], f32)
            nc.sync.dma_start(cb[:], cb_d[:])
            # whole-run xv/ones stream: one small DMA (dispatched after the
            # first tile's din so the first matmuls aren't queued behind it)
            aux_sb = cpool.tile([3, ns], bf16)

            # warmup stream: absorbs the cw DMA wait and accumulates the ~3us
            # of continuous PE activity that ramps the HAM clock to full speed
            # before the first real matmul
            warm = psx.tile([128, T], f32, tag="x")
            for _ in range(16):
                nc.tensor.matmul(warm[:, 0:256], cw[0:3, CO_WP:CO_WP + 128],
                                 cw[0:3, 0:256], start=True, stop=True)

            stash = {}

            def stage_a(t, first=False):
                w = T if t < nt else tw
                src = tile_view(din_d, t, 6)
                din_t = dinp.tile([128, 6, T], bf16, tag="din")
                nc.sync.dma_start(din_t[:, :, 0:w], src[:])
                if first:
                    nc.sync.dma_start(aux_sb[:], aux_d[:])

                # x_ps: partitions 0:64 e_v, 64:128 a_v, biases included via
                # the aux ones-row
                x_ps = psx.tile([128, T], f32, tag="x")
                for k0 in range(0, w, TS):
                    ksl = slice(k0, min(k0 + TS, w))
                    for c in range(4):
                        nc.tensor.matmul(
                            x_ps[:, ksl],
                            cw[:, CO_WHID + 128 * c:CO_WHID + 128 * (c + 1)],
                            din_t[:, c, ksl], start=(c == 0), stop=False)
                    nc.tensor.matmul(
                        x_ps[:, ksl], cw[0:3, CO_WP:CO_WP + 128],
                        aux_sb[0:3, t * T + ksl.start:t * T + ksl.stop],
                        start=False, stop=True)

                # x = relu(x_ps) on the DVE, rounded to bf16 for the gate matmuls
                x_sb = xsbp.tile([128, T], bf16, tag="x_sb")
                nc.vector.tensor_scalar_max(x_sb[:, 0:w], x_ps[:, 0:w], 0.0)
                stash[t] = (din_t, x_sb)

            def stage_b(t, nsplit=1, final=False):
                din_t, x_sb = stash.pop(t)
                hcn = hcnp.tile([128, 2, T], bf16, tag="hcn")
                dst = tile_view(out_d, t, 2)
                w = (T if t < nt else tw) // nsplit
                for h in range(nsplit):
                    hsl = slice(h * w, (h + 1) * w)
                    gact = [None] * 4
                    # per gate chunk j: g_ps_j = W_hh.T_j @ hv.T + W_ih.T_j @ x.
                    # f first so t1 = f*cv can start early on the DVE.
                    for j in (1, 0, 2, 3):
                        gp = psg.tile([128, T], f32, tag="g")
                        for k0 in range(h * w, (h + 1) * w, TS):
                            ksl = slice(k0, min(k0 + TS, (h + 1) * w))
                            nc.tensor.matmul(
                                gp[:, ksl],
                                cw[:, CO_WHH + 128 * j:CO_WHH + 128 * (j + 1)],
                                din_t[:, 4, ksl], start=True, stop=False)
                            nc.tensor.matmul(
                                gp[:, ksl],
                                cw[:, CO_WIH + 128 * j:CO_WIH + 128 * (j + 1)],
                                x_sb[:, ksl], start=False, stop=True)
                        ga = gactp.tile([128, T], bf16, tag=f"g{j}")
                        gact[j] = ga
                        nc.scalar.activation(ga[:, hsl], gp[:, hsl], GATE_FUNCS[j],
                                             bias=cb[:, j:j + 1])
                        if j == 1:
                            # t1 = f * cv while the other gates are in flight
                            t1 = tmpp.tile([128, T], bf16, tag="t1")
                            nc.vector.tensor_tensor(t1[:, hsl], ga[:, hsl],
                                                    din_t[:, 5, hsl], ALU.mult)
                    i_s, f_s, g_t, o_s = gact

                    t2 = tmpp.tile([128, T], bf16, tag="t2")
                    th = tmpp.tile([128, T], bf16, tag="th")
                    nc.vector.tensor_tensor(t2[:, hsl], i_s[:, hsl], g_t[:, hsl],
                                            ALU.mult)
                    nc.vector.tensor_tensor(hcn[:, 1, hsl], t1[:, hsl], t2[:, hsl],
                                            ALU.add)
                    nc.scalar.activation(th[:, hsl], hcn[:, 1, hsl], AF.Tanh)
                    nc.vector.tensor_tensor(hcn[:, 0, hsl], o_s[:, hsl], th[:, hsl],
                                            ALU.mult)
                    # out goes via the GPSIMD software DGE so the sync HW queue
                    # carries only the din stream; the final tile's outs go via
                    # sync (no din left there) to skip the slow SWDGE drain
                    eng = nc.sync if final else nc.gpsimd
                    eng.dma_start(dst[:, :, hsl], hcn[:, :, hsl])

            # tail tile first: its half-size din lands soonest, so the PE
            # starts real work earlier; full tiles stream behind it
            order = ([nt] if tw else []) + list(range(nt))
            for i in range(len(order) + 1):
                if i < len(order):
                    stage_a(order[i], first=(i == 0))
                if i >= 1:
                    t = order[i - 1]
                    last = i - 1 == len(order) - 1
                    stage_b(t, nsplit=2 if last and t < nt else 1, final=last)

    nc.finalize()
    return nc


def _stage_weights(W_pos, b_pos, W_hid, b_hid, W_ih, b_ih, W_hh, b_hh):
    cw = np.zeros((128, CWF), dtype=np.float32)
    whid_t = np.ascontiguousarray(W_hid.T)          # [512, 64]
    for c in range(4):
        cw[:, CO_WHID + 128 * c + 64:CO_WHID + 128 * (c + 1)] = whid_t[128 * c:128 * (c + 1)]
    cw[:, CO_WIH:CO_WIH + 512] = W_ih.T             # [128, 512]
    cw[:, CO_WHH:CO_WHH + 512] = W_hh.T
    cw[0:2, CO_WP:CO_WP + 64] = W_pos.T             # [2, 64]
    cw[2, CO_WP:CO_WP + 64] = b_pos
    cw[2, CO_WP + 64:CO_WP + 128] = b_hid
    cb = np.zeros((128, 4), dtype=np.float32)
    cb[:, :] = (b_ih + b_hh).reshape(4, 128).T
    return cw.astype(BF16), cb


def _stage_inputs(Hv_t, hvv_t, xv_t, hv_tm1, cv_tm1, act, ncap, weights):
    """Gather active rows, pad to ncap total, stage feature-major bf16."""
    cw, cb = _stage_weights(**weights)
    actp = np.pad(act, (0, ncap - act.size), mode="edge") if act.size < ncap else act
    ns = ncap // NCORES
    nt, tw = ns // T, ns % T
    in_maps = []
    for s in range(NCORES):
        idx = actp[s * ns:(s + 1) * ns]
        buf = np.empty((ns, 768), dtype=np.float32)
        buf[:, 0:256] = hvv_t[idx]
        buf[:, 256:512] = Hv_t[idx]
        buf[:, 512:640] = hv_tm1[idx]
        buf[:, 640:768] = cv_tm1[idx]
        # per tile: din[p, c, n] = buf_tile[n, c*128+p]
        bufb = buf.astype(BF16)
        din = np.empty((128, ns * 6), dtype=BF16)
        din[:, 0:nt * 6 * T] = (bufb[0:nt * T].reshape(nt, T, 6, 128)
                                .transpose(3, 0, 2, 1).reshape(128, nt * 6 * T))
        if tw:
            din[:, nt * 6 * T:] = (bufb[nt * T:].reshape(tw, 6, 128)
                                   .transpose(2, 1, 0).reshape(128, 6 * tw))
        aux = np.empty((3, ns), dtype=np.float32)
        aux[0:2] = xv_t[idx].T
        aux[2] = 1.0
        in_maps.append(dict(din=din, aux=aux.astype(BF16), cw=cw, cb=cb))
    return in_maps, actp


def run(inputs, trace=False):
    """Stage, run on 8 cores, unstage. Returns ((hv_t, cv_t), BassKernelResults)."""
    inputs = {k: np.asarray(v) for k, v in inputs.items()}
    weights = {k: inputs[k] for k in ["W_pos", "b_pos", "W_hid", "b_hid",
                                      "W_ih", "b_ih", "W_hh", "b_hh"]}
    act = np.flatnonzero(inputs["ts_mask"][:, 0] == 1)
    if act.size == 0:
        return (inputs["hv_tm1"].copy(), inputs["cv_tm1"].copy()), None
    grain = NCORES * 128
    ncap = -(-act.size // grain) * grain
    ns = ncap // NCORES
    nt, tw = ns // T, ns % T

    in_maps, actp = _stage_inputs(inputs["Hv_t"], inputs["hvv_t"], inputs["xv_t"],
                                  inputs["hv_tm1"], inputs["cv_tm1"], act, ncap,
                                  weights)
    if (nt, tw) not in _cached:
        _cached[(nt, tw)] = build_nc(nt, tw)
    res = run_bass_kernel_spmd(_cached[(nt, tw)], in_maps,
                               core_ids=list(range(NCORES)), trace=trace)
    hv_out = inputs["hv_tm1"].astype(np.float32, copy=True)
    cv_out = inputs["cv_tm1"].astype(np.float32, copy=True)
    na = act.size
    for s in range(NCORES):
        lo, hi = s * ns, (s + 1) * ns
        if lo >= na:
            break
        o = res.results[s]["hc_out"]
        hc = np.empty((2, 128, ns), dtype=BF16)
        hc[:, :, 0:nt * T] = (o[:, 0:nt * 2 * T].reshape(128, nt, 2, T)
                              .transpose(2, 0, 1, 3).reshape(2, 128, nt * T))
        if tw:
            hc[:, :, nt * T:] = (o[:, nt * 2 * T:].reshape(128, 2, tw)
                                 .transpose(1, 0, 2))
        n_keep = min(hi, na) - lo
        hv_out[act[lo:lo + n_keep]] = hc[0].T[:n_keep].astype(np.float32)
        cv_out[act[lo:lo + n_keep]] = hc[1].T[:n_keep].astype(np.float32)
    return (hv_out, cv_out), res


def kernel(**inputs):
    out, _ = run(inputs, trace=False)
    return out


# revision 49
# speedup vs baseline: 1.1909x; 1.0095x over previous
"""Trainium2 Bass kernel for nn_NodeRNN (masked single-step LSTM over N nodes).

Strategy: the reference computes the LSTM step everywhere and then keeps the
old state for inactive nodes (ts_mask != 1). Equivalently: gather the active
rows, run the LSTM step on just those, scatter back. The gather/scatter is
pure data routing, done host-side during staging (where the baseline already
transposes); only active nodes ever touch the device. That halves HBM traffic
and every engine's work.

All per-node data is staged FEATURE-MAJOR and in bf16 (tolerance is 2e-2;
bf16 keeps us ~3 orders of magnitude under it), packed into ONE interleaved
dram stream per core laid out [128, NT, 6, T] so each tile is a single DMA of
128 x 12KB contiguous descriptors:
    chunk 0..1: hvv.T      chunk 2..3: Hv.T      chunk 4: hv.T   chunk 5: cv.T

Per 1024-node tile (two 512-node matmul subtiles per PSUM bank):
  x_ps  = [W_pos @ xv.T + b_pos ; W_hid @ X.T + b_hid]   (PE; biases folded
          into the matmul via a ones-row in the aux stream)
  x     = relu(x_ps)                                     (DVE max, -> bf16)
  gates = W_ih @ x + W_hh @ hv.T (+ fused bias via ACT)  (PE)
  i,f,o = sigmoid, g = tanh                              (ACT, -> bf16)
  c_new = f*cv + i*g ; h_new = o*tanh(c_new)             (DVE + one ACT tanh)
Outputs leave as bf16 [128, NT, 2, T]; host scatters them back into f32
copies of hv_tm1/cv_tm1 (inactive rows therefore stay bit-exact).
Emission is software-pipelined (stage A of tile t+1 before stage B of tile t).
"""
import sys

sys.path.insert(0, "/opt/trn_rl_repo")

import numpy as np
import ml_dtypes

import concourse.bacc as bacc
import concourse.tile as tile
from concourse import mybir
from concourse.bass_utils import run_bass_kernel_spmd

f32 = mybir.dt.float32
bf16 = mybir.dt.bfloat16
AF = mybir.ActivationFunctionType
ALU = mybir.AluOpType
BF16 = ml_dtypes.bfloat16

N = 262144
NCORES = 8
T = 1024                  # nodes per tile (DMA + elementwise granularity)
TS = 512                  # matmul subtile (PSUM bank = 512 f32)
EMBED = 64
EDGE_H = 256
NODE_H = 128

# weight block layout: [128, CWF] bf16, free-dim offsets
CO_WHID = 0               # 4 chunks x 128 cols; cols 64:128 of chunk c = W_hid.T chunk
CO_WIH = 512              # W_ih.T [128, 512]
CO_WHH = 1024             # W_hh.T [128, 512]
CO_WP = 1536              # [3, 128]: rows 0:2 = W_pos.T | 0, row 2 = concat(b_pos, b_hid)
CWF = 1664

GATE_FUNCS = [AF.Sigmoid, AF.Sigmoid, AF.Tanh, AF.Sigmoid]  # i, f, g, o

_cached = {}


def build_nc(nt, tw):
    """nt full tiles of T nodes plus one tail tile of tw nodes (multiple of
    128, 0..T-128; the tail is processed FIRST so its small din fill lets the
    PE start early)."""
    ns = nt * T + tw
    nc = bacc.Bacc(target_bir_lowering=False)
    din_d = nc.dram_tensor("din", [128, ns * 6], bf16, kind="ExternalInput")
    aux_d = nc.dram_tensor("aux", [3, ns], bf16, kind="ExternalInput")
    cw_d = nc.dram_tensor("cw", [128, CWF], bf16, kind="ExternalInput")
    cb_d = nc.dram_tensor("cb", [128, 4], f32, kind="ExternalInput")
    out_d = nc.dram_tensor("hc_out", [128, ns * 2], bf16, kind="ExternalOutput")

    def tile_view(dram, t, c):
        w = T if t < nt else tw
        off = t * c * T
        return dram[:, off:off + c * w].rearrange("p (c n) -> p c n", c=c)

    with tile.TileContext(nc) as tc:
        with (
            tc.tile_pool(name="const", bufs=1) as cpool,
            tc.tile_pool(name="din", bufs=6) as dinp,
            tc.tile_pool(name="xsb", bufs=3) as xsbp,
            tc.tile_pool(name="gact", bufs=3) as gactp,
            tc.tile_pool(name="tmp", bufs=3) as tmpp,
            tc.tile_pool(name="hcn", bufs=3) as hcnp,
            tc.tile_pool(name="ps_x", bufs=1, space="PSUM") as psx,
            tc.tile_pool(name="ps_g", bufs=3, space="PSUM") as psg,
        ):
            cw = cpool.tile([128, CWF], bf16)
            nc.sync.dma_start(cw[:], cw_d[:])
            cb = cpool.tile([128, 4], f32)
            nc.sync.dma_start(cb[:], cb_d[:])
            # whole-run xv/ones stream: one small DMA (dispatched after the
            # first tile's din so the first matmuls aren't queued behind it)
            aux_sb = cpool.tile([3, ns], bf16)

            # warmup stream: absorbs the cw DMA wait and accumulates the ~3us
            # of continuous PE activity that ramps the HAM clock to full speed
            # before the first real matmul
            warm = psx.tile([128, T], f32, tag="x")
            for _ in range(16):
                nc.tensor.matmul(warm[:, 0:256], cw[0:3, CO_WP:CO_WP + 128],
                                 cw[0:3, 0:256], start=True, stop=True)

            stash = {}

            def stage_a(t, first=False):
                w = T if t < nt else tw
                src = tile_view(din_d, t, 6)
                din_t = dinp.tile([128, 6, T], bf16, tag="din")
                nc.sync.dma_start(din_t[:, :, 0:w], src[:])
                if first:
                    nc.sync.dma_start(aux_sb[:], aux_d[:])

                # x_ps: partitions 0:64 e_v, 64:128 a_v, biases included via
                # the aux ones-row
                x_ps = psx.tile([128, T], f32, tag="x")
                for k0 in range(0, w, TS):
                    ksl = slice(k0, min(k0 + TS, w))
                    for c in range(4):
                        nc.tensor.matmul(
                            x_ps[:, ksl],
                            cw[:, CO_WHID + 128 * c:CO_WHID + 128 * (c + 1)],
                            din_t[:, c, ksl], start=(c == 0), stop=False)
                    nc.tensor.matmul(
                        x_ps[:, ksl], cw[0:3, CO_WP:CO_WP + 128],
                        aux_sb[0:3, t * T + ksl.start:t * T + ksl.stop],
                        start=False, stop=True)

                # x = relu(x_ps) on the DVE, rounded to bf16 for the gate matmuls
                x_sb = xsbp.tile([128, T], bf16, tag="x_sb")
                nc.vector.tensor_scalar_max(x_sb[:, 0:w], x_ps[:, 0:w], 0.0)
                stash[t] = (din_t, x_sb)

            def stage_b(t, nsplit=1, final=False):
                din_t, x_sb = stash.pop(t)
                hcn = hcnp.tile([128, 2, T], bf16, tag="hcn")
                dst = tile_view(out_d, t, 2)
                w = (T if t < nt else tw) // nsplit
                for h in range(nsplit):
                    hsl = slice(h * w, (h + 1) * w)
                    gact = [None] * 4
                    # per gate chunk j: g_ps_j = W_hh.T_j @ hv.T + W_ih.T_j @ x.
                    # f first so t1 = f*cv can start early on the DVE.
                    for j in (1, 0, 2, 3):
                        gp = psg.tile([128, T], f32, tag="g")
                        for k0 in range(h * w, (h + 1) * w, TS):
                            ksl = slice(k0, min(k0 + TS, (h + 1) * w))
                            nc.tensor.matmul(
                                gp[:, ksl],
                                cw[:, CO_WHH + 128 * j:CO_WHH + 128 * (j + 1)],
                                din_t[:, 4, ksl], start=True, stop=False)
                            nc.tensor.matmul(
                                gp[:, ksl],
                                cw[:, CO_WIH + 128 * j:CO_WIH + 128 * (j + 1)],
                                x_sb[:, ksl], start=False, stop=True)
                        ga = gactp.tile([128, T], bf16, tag=f"g{j}")
                        gact[j] = ga
                        nc.scalar.activation(ga[:, hsl], gp[:, hsl], GATE_FUNCS[j],
                                             bias=cb[:, j:j + 1])
                        if j == 1:
                            # t1 = f * cv while the other gates are in flight
                            t1 = tmpp.tile([128, T], bf16, tag="t1")
                            nc.vector.tensor_tensor(t1[:, hsl], ga[:, hsl],
                                                    din_t[:, 5, hsl], ALU.mult)
                    i_s, f_s, g_t, o_s = gact

                    t2 = tmpp.tile([128, T], bf16, tag="t2")
                    th = tmpp.tile([128, T], bf16, tag="th")
                    nc.vector.tensor_tensor(t2[:, hsl], i_s[:, hsl], g_t[:, hsl],
                                            ALU.mult)
                    nc.vector.tensor_tensor(hcn[:, 1, hsl], t1[:, hsl], t2[:, hsl],
                                            ALU.add)
                    nc.scalar.activation(th[:, hsl], hcn[:, 1, hsl], AF.Tanh)
                    nc.vector.tensor_tensor(hcn[:, 0, hsl], o_s[:, hsl], th[:, hsl],
                                            ALU.mult)
                    # out goes via the GPSIMD software DGE so the sync HW queue
                    # carries only the din stream; the final tile's outs go via
                    # sync (no din left there) to skip the slow SWDGE drain
                    eng = nc.sync if final else nc.gpsimd
                    eng.dma_start(dst[:, :, hsl], hcn[:, :, hsl])

            # tail tile first: its half-size din lands soonest, so the PE
            # starts real work earlier; full tiles stream behind it
            order = ([nt] if tw else []) + list(range(nt))
            for i in range(len(order) + 1):
                if i < len(order):
                    stage_a(order[i], first=(i == 0))
                if i >= 1:
                    t = order[i - 1]
                    last = i - 1 == len(order) - 1
                    stage_b(t, nsplit=2 if last and t < nt else 1, final=last)

    nc.finalize()
    return nc


def _stage_weights(W_pos, b_pos, W_hid, b_hid, W_ih, b_ih, W_hh, b_hh):
    cw = np.zeros((128, CWF), dtype=np.float32)
    whid_t = np.ascontiguousarray(W_hid.T)          # [512, 64]
    for c in range(4):
        cw[:, CO_WHID + 128 * c + 64:CO_WHID + 128 * (c + 1)] = whid_t[128 * c:128 * (c + 1)]
    cw[:, CO_WIH:CO_WIH + 512] = W_ih.T             # [128, 512]
    cw[:, CO_WHH:CO_WHH + 512] = W_hh.T
    cw[0:2, CO_WP:CO_WP + 64] = W_pos.T             # [2, 64]
    cw[2, CO_WP:CO_WP + 64] = b_pos
    cw[2, CO_WP + 64:CO_WP + 128] = b_hid
    cb = np.zeros((128, 4), dtype=np.float32)
    cb[:, :] = (b_ih + b_hh).reshape(4, 128).T
    return cw.astype(BF16), cb


def _stage_inputs(Hv_t, hvv_t, xv_t, hv_tm1, cv_tm1, act, ncap, weights):
    """Gather active rows, pad to ncap total, stage feature-major bf16."""
    cw, cb = _stage_weights(**weights)
    actp = np.pad(act, (0, ncap - act.size), mode="edge") if act.size < ncap else act
    ns = ncap // NCORES
    nt, tw = ns // T, ns % T
    in_maps = []
    for s in range(NCORES):
        idx = actp[s * ns:(s + 1) * ns]
        buf = np.empty((ns, 768), dtype=np.float32)
        buf[:, 0:256] = hvv_t[idx]
        buf[:, 256:512] = Hv_t[idx]
        buf[:, 512:640] = hv_tm1[idx]
        buf[:, 640:768] = cv_tm1[idx]
        # per tile: din[p, c, n] = buf_tile[n, c*128+p]
        bufb = buf.astype(BF16)
        din = np.empty((128, ns * 6), dtype=BF16)
        din[:, 0:nt * 6 * T] = (bufb[0:nt * T].reshape(nt, T, 6, 128)
                                .transpose(3, 0, 2, 1).reshape(128, nt * 6 * T))
        if tw:
            din[:, nt * 6 * T:] = (bufb[nt * T:].reshape(tw, 6, 128)
                                   .transpose(2, 1, 0).reshape(128, 6 * tw))
        aux = np.empty((3, ns), dtype=np.float32)
        aux[0:2] = xv_t[idx].T
        aux[2] = 1.0
        in_maps.append(dict(din=din, aux=aux.astype(BF16), cw=cw, cb=cb))
    return in_maps, actp


def run(inputs, trace=False):
    """Stage, run on 8 cores, unstage. Returns ((hv_t, cv_t), BassKernelResults)."""
    inputs = {k: np.asarray(v) for k, v in inputs.items()}
    weights = {k: inputs[k] for k in ["W_pos", "b_pos", "W_hid", "b_hid",
                                      "W_ih", "b_ih", "W_hh", "b_hh"]}
    act = np.flatnonzero(inputs["ts_mask"][:, 0] == 1)
    if act.size == 0:
        return (inputs["hv_tm1"].copy(), inputs["cv_tm1"].copy()), None
    grain = NCORES * 128
    ncap = -(-act.size // grain) * grain
    ns = ncap // NCORES
    nt, tw = ns // T, ns % T

    in_maps, actp = _stage_inputs(inputs["Hv_t"], inputs["hvv_t"], inputs["xv_t"],
                                  inputs["hv_tm1"], inputs["cv_tm1"], act, ncap,
                                  weights)
    if (nt, tw) not in _cached:
        _cached[(nt, tw)] = build_nc(nt, tw)
    res = run_bass_kernel_spmd(_cached[(nt, tw)], in_maps,
                               core_ids=list(range(NCORES)), trace=trace)
    hv_out = inputs["hv_tm1"].astype(np.float32, copy=True)
    cv_out = inputs["cv_tm1"].astype(np.float32, copy=True)
    na = act.size
    for s in range(NCORES):
        lo, hi = s * ns, (s + 1) * ns
        if lo >= na:
            break
        o = res.results[s]["hc_out"]
        hc = np.empty((2, 128, ns), dtype=BF16)
        hc[:, :, 0:nt * T] = (o[:, 0:nt * 2 * T].reshape(128, nt, 2, T)
                              .transpose(2, 0, 1, 3).reshape(2, 128, nt * T))
        if tw:
            hc[:, :, nt * T:] = (o[:, nt * 2 * T:].reshape(128, 2, tw)
                                 .transpose(1, 0, 2))
        n_keep = min(hi, na) - lo
        hv_out[act[lo:lo + n_keep]] = hc[0].T[:n_keep].astype(np.float32)
        cv_out[act[lo:lo + n_keep]] = hc[1].T[:n_keep].astype(np.float32)
    return (hv_out, cv_out), res


def kernel(**inputs):
    out, _ = run(inputs, trace=False)
    return out


# revision 50
# speedup vs baseline: 1.1917x; 1.0007x over previous
"""Trainium2 Bass kernel for nn_NodeRNN (masked single-step LSTM over N nodes).

Strategy: the reference computes the LSTM step everywhere and then keeps the
old state for inactive nodes (ts_mask != 1). Equivalently: gather the active
rows, run the LSTM step on just those, scatter back. The gather/scatter is
pure data routing, done host-side during staging (where the baseline already
transposes); only active nodes ever touch the device. That halves HBM traffic
and every engine's work.

All per-node data is staged FEATURE-MAJOR and in bf16 (tolerance is 2e-2;
bf16 keeps us ~3 orders of magnitude under it), packed into ONE interleaved
dram stream per core laid out [128, NT, 6, T] so each tile is a single DMA of
128 x 12KB contiguous descriptors:
    chunk 0..1: hvv.T      chunk 2..3: Hv.T      chunk 4: hv.T   chunk 5: cv.T

Per 1024-node tile (two 512-node matmul subtiles per PSUM bank):
  x_ps  = [W_pos @ xv.T + b_pos ; W_hid @ X.T + b_hid]   (PE; biases folded
          into the matmul via a ones-row in the aux stream)
  x     = relu(x_ps)                                     (DVE max, -> bf16)
  gates = W_ih @ x + W_hh @ hv.T (+ fused bias via ACT)  (PE)
  i,f,o = sigmoid, g = tanh                              (ACT, -> bf16)
  c_new = f*cv + i*g ; h_new = o*tanh(c_new)             (DVE + one ACT tanh)
Outputs leave as bf16 [128, NT, 2, T]; host scatters them back into f32
copies of hv_tm1/cv_tm1 (inactive rows therefore stay bit-exact).
Emission is software-pipelined (stage A of tile t+1 before stage B of tile t).
"""
import sys

sys.path.insert(0, "/opt/trn_rl_repo")

import numpy as np
import ml_dtypes

import concourse.bacc as bacc
import concourse.tile as tile
from concourse import mybir
from concourse.bass_utils import run_bass_kernel_spmd

f32 = mybir.dt.float32
bf16 = mybir.dt.bfloat16
AF = mybir.ActivationFunctionType
ALU = mybir.AluOpType
BF16 = ml_dtypes.bfloat16

N = 262144
NCORES = 8
T = 1024                  # nodes per tile (DMA + elementwise granularity)
TS = 512                  # matmul subtile (PSUM bank = 512 f32)
EMBED = 64
EDGE_H = 256
NODE_H = 128

# weight block layout: [128, CWF] bf16, free-dim offsets
CO_WHID = 0               # 4 chunks x 128 cols; cols 64:128 of chunk c = W_hid.T chunk
CO_WIH = 512              # W_ih.T [128, 512]
CO_WHH = 1024             # W_hh.T [128, 512]
CO_WP = 1536              # [3, 128]: rows 0:2 = W_pos.T | 0, row 2 = concat(b_pos, b_hid)
CWF = 1664

GATE_FUNCS = [AF.Sigmoid, AF.Sigmoid, AF.Tanh, AF.Sigmoid]  # i, f, g, o

_cached = {}


def build_nc(nt, tw):
    """nt full tiles of T nodes plus one tail tile of tw nodes (multiple of
    128, 0..T-128; the tail is processed FIRST so its small din fill lets the
    PE start early)."""
    ns = nt * T + tw
    nc = bacc.Bacc(target_bir_lowering=False)
    din_d = nc.dram_tensor("din", [128, ns * 6], bf16, kind="ExternalInput")
    aux_d = nc.dram_tensor("aux", [3, ns], bf16, kind="ExternalInput")
    cw_d = nc.dram_tensor("cw", [128, CWF], bf16, kind="ExternalInput")
    cb_d = nc.dram_tensor("cb", [128, 4], f32, kind="ExternalInput")
    out_d = nc.dram_tensor("hc_out", [128, ns * 2], bf16, kind="ExternalOutput")

    def tile_view(dram, t, c):
        w = T if t < nt else tw
        off = t * c * T
        return dram[:, off:off + c * w].rearrange("p (c n) -> p c n", c=c)

    with tile.TileContext(nc) as tc:
        with (
            tc.tile_pool(name="const", bufs=1) as cpool,
            tc.tile_pool(name="din", bufs=6) as dinp,
            tc.tile_pool(name="xsb", bufs=3) as xsbp,
            tc.tile_pool(name="gact", bufs=3) as gactp,
            tc.tile_pool(name="tmp", bufs=3) as tmpp,
            tc.tile_pool(name="hcn", bufs=3) as hcnp,
            tc.tile_pool(name="ps_x", bufs=1, space="PSUM") as psx,
            tc.tile_pool(name="ps_g", bufs=3, space="PSUM") as psg,
        ):
            cw = cpool.tile([128, CWF], bf16)
            nc.sync.dma_start(cw[:], cw_d[:])
            cb = cpool.tile([128, 4], f32)
            nc.sync.dma_start(cb[:], cb_d[:])
            # whole-run xv/ones stream: one small DMA (dispatched after the
            # first tile's din so the first matmuls aren't queued behind it)
            aux_sb = cpool.tile([3, ns], bf16)

            # warmup stream: absorbs the cw DMA wait and accumulates the ~3us
            # of continuous PE activity that ramps the HAM clock to full speed
            # before the first real matmul
            warm = psx.tile([128, T], f32, tag="x")
            for _ in range(16):
                nc.tensor.matmul(warm[:, 0:256], cw[0:3, CO_WP:CO_WP + 128],
                                 cw[0:3, 0:256], start=True, stop=True)

            stash = {}

            def stage_a(t, first=False):
                w = T if t < nt else tw
                src = tile_view(din_d, t, 6)
                din_t = dinp.tile([128, 6, T], bf16, tag="din")
                nc.sync.dma_start(din_t[:, :, 0:w], src[:])
                if first:
                    nc.sync.dma_start(aux_sb[:], aux_d[:])

                # x_ps: partitions 0:64 e_v, 64:128 a_v, biases included via
                # the aux ones-row
                x_ps = psx.tile([128, T], f32, tag="x")
                for k0 in range(0, w, TS):
                    ksl = slice(k0, min(k0 + TS, w))
                    for c in range(4):
                        nc.tensor.matmul(
                            x_ps[:, ksl],
                            cw[:, CO_WHID + 128 * c:CO_WHID + 128 * (c + 1)],
                            din_t[:, c, ksl], start=(c == 0), stop=False)
                    nc.tensor.matmul(
                        x_ps[:, ksl], cw[0:3, CO_WP:CO_WP + 128],
                        aux_sb[0:3, t * T + ksl.start:t * T + ksl.stop],
                        start=False, stop=True)

                # x = relu(x_ps) on the DVE, rounded to bf16 for the gate matmuls
                x_sb = xsbp.tile([128, T], bf16, tag="x_sb")
                nc.vector.tensor_scalar_max(x_sb[:, 0:w], x_ps[:, 0:w], 0.0)
                stash[t] = (din_t, x_sb)

            stash2 = {}

            def stage_b1(t):
                """Gate matmuls + gate activations + c_new. tanh(c)/h/out run
                one pipeline iteration later (stage_b2) so the ACT engine can
                start the next tile's gates instead of idling until the DVE
                finishes c."""
                din_t, x_sb = stash.pop(t)
                w = T if t < nt else tw
                hcn = hcnp.tile([128, 2, T], bf16, tag="hcn")
                gact = [None] * 4
                # per gate chunk j: g_ps_j = W_hh.T_j @ hv.T + W_ih.T_j @ x.
                # f first so t1 = f*cv can start early on the DVE.
                for j in (1, 0, 2, 3):
                    gp = psg.tile([128, T], f32, tag="g")
                    for k0 in range(0, w, TS):
                        ksl = slice(k0, min(k0 + TS, w))
                        nc.tensor.matmul(
                            gp[:, ksl],
                            cw[:, CO_WHH + 128 * j:CO_WHH + 128 * (j + 1)],
                            din_t[:, 4, ksl], start=True, stop=False)
                        nc.tensor.matmul(
                            gp[:, ksl],
                            cw[:, CO_WIH + 128 * j:CO_WIH + 128 * (j + 1)],
                            x_sb[:, ksl], start=False, stop=True)
                    ga = gactp.tile([128, T], bf16, tag=f"g{j}")
                    gact[j] = ga
                    nc.scalar.activation(ga[:, 0:w], gp[:, 0:w], GATE_FUNCS[j],
                                         bias=cb[:, j:j + 1])
                    if j == 1:
                        # t1 = f * cv while the other gates are in flight
                        t1 = tmpp.tile([128, T], bf16, tag="t1")
                        nc.vector.tensor_tensor(t1[:, 0:w], ga[:, 0:w],
                                                din_t[:, 5, 0:w], ALU.mult)
                i_s, f_s, g_t, o_s = gact

                t2 = tmpp.tile([128, T], bf16, tag="t2")
                nc.vector.tensor_tensor(t2[:, 0:w], i_s[:, 0:w], g_t[:, 0:w],
                                        ALU.mult)
                nc.vector.tensor_tensor(hcn[:, 1, 0:w], t1[:, 0:w], t2[:, 0:w],
                                        ALU.add)
                stash2[t] = (hcn, o_s)

            def stage_b2(t, final=False):
                hcn, o_s = stash2.pop(t)
                w = T if t < nt else tw
                dst = tile_view(out_d, t, 2)
                th = tmpp.tile([128, T], bf16, tag="th")
                nc.scalar.activation(th[:, 0:w], hcn[:, 1, 0:w], AF.Tanh)
                nc.vector.tensor_tensor(hcn[:, 0, 0:w], o_s[:, 0:w], th[:, 0:w],
                                        ALU.mult)
                # out goes via the GPSIMD software DGE so the sync HW queue
                # carries only the din stream; the final tile's out goes via
                # sync (no din left there) to skip the slow SWDGE drain
                eng = nc.sync if final else nc.gpsimd
                eng.dma_start(dst[:, :, 0:w], hcn[:, :, 0:w])

            # tail tile first: its half-size din lands soonest, so the PE
            # starts real work earlier; full tiles stream behind it
            order = ([nt] if tw else []) + list(range(nt))
            for i in range(len(order) + 2):
                if i < len(order):
                    stage_a(order[i], first=(i == 0))
                if 1 <= i <= len(order):
                    stage_b1(order[i - 1])
                if i >= 2:
                    stage_b2(order[i - 2], final=(i - 2 == len(order) - 1))

    nc.finalize()
    return nc


def _stage_weights(W_pos, b_pos, W_hid, b_hid, W_ih, b_ih, W_hh, b_hh):
    cw = np.zeros((128, CWF), dtype=np.float32)
    whid_t = np.ascontiguousarray(W_hid.T)          # [512, 64]
    for c in range(4):
        cw[:, CO_WHID + 128 * c + 64:CO_WHID + 128 * (c + 1)] = whid_t[128 * c:128 * (c + 1)]
    cw[:, CO_WIH:CO_WIH + 512] = W_ih.T             # [128, 512]
    cw[:, CO_WHH:CO_WHH + 512] = W_hh.T
    cw[0:2, CO_WP:CO_WP + 64] = W_pos.T             # [2, 64]
    cw[2, CO_WP:CO_WP + 64] = b_pos
    cw[2, CO_WP + 64:CO_WP + 128] = b_hid
    cb = np.zeros((128, 4), dtype=np.float32)
    cb[:, :] = (b_ih + b_hh).reshape(4, 128).T
    return cw.astype(BF16), cb


def _stage_inputs(Hv_t, hvv_t, xv_t, hv_tm1, cv_tm1, act, ncap, weights):
    """Gather active rows, pad to ncap total, stage feature-major bf16."""
    cw, cb = _stage_weights(**weights)
    actp = np.pad(act, (0, ncap - act.size), mode="edge") if act.size < ncap else act
    ns = ncap // NCORES
    nt, tw = ns // T, ns % T
    in_maps = []
    for s in range(NCORES):
        idx = actp[s * ns:(s + 1) * ns]
        buf = np.empty((ns, 768), dtype=np.float32)
        buf[:, 0:256] = hvv_t[idx]
        buf[:, 256:512] = Hv_t[idx]
        buf[:, 512:640] = hv_tm1[idx]
        buf[:, 640:768] = cv_tm1[idx]
        # per tile: din[p, c, n] = buf_tile[n, c*128+p]
        bufb = buf.astype(BF16)
        din = np.empty((128, ns * 6), dtype=BF16)
        din[:, 0:nt * 6 * T] = (bufb[0:nt * T].reshape(nt, T, 6, 128)
                                .transpose(3, 0, 2, 1).reshape(128, nt * 6 * T))
        if tw:
            din[:, nt * 6 * T:] = (bufb[nt * T:].reshape(tw, 6, 128)
                                   .transpose(2, 1, 0).reshape(128, 6 * tw))
        aux = np.empty((3, ns), dtype=np.float32)
        aux[0:2] = xv_t[idx].T
        aux[2] = 1.0
        in_maps.append(dict(din=din, aux=aux.astype(BF16), cw=cw, cb=cb))
    return in_maps, actp


def run(inputs, trace=False):
    """Stage, run on 8 cores, unstage. Returns ((hv_t, cv_t), BassKernelResults)."""
    inputs = {k: np.asarray(v) for k, v in inputs.items()}
    weights = {k: inputs[k] for k in ["W_pos", "b_pos", "W_hid", "b_hid",
                                      "W_ih", "b_ih", "W_hh", "b_hh"]}
    act = np.flatnonzero(inputs["ts_mask"][:, 0] == 1)
    if act.size == 0:
        return (inputs["hv_tm1"].copy(), inputs["cv_tm1"].copy()), None
    grain = NCORES * 128
    ncap = -(-act.size // grain) * grain
    ns = ncap // NCORES
    nt, tw = ns // T, ns % T

    in_maps, actp = _stage_inputs(inputs["Hv_t"], inputs["hvv_t"], inputs["xv_t"],
                                  inputs["hv_tm1"], inputs["cv_tm1"], act, ncap,
                                  weights)
    if (nt, tw) not in _cached:
        _cached[(nt, tw)] = build_nc(nt, tw)
    res = run_bass_kernel_spmd(_cached[(nt, tw)], in_maps,
                               core_ids=list(range(NCORES)), trace=trace)
    hv_out = inputs["hv_tm1"].astype(np.float32, copy=True)
    cv_out = inputs["cv_tm1"].astype(np.float32, copy=True)
    na = act.size
    for s in range(NCORES):
        lo, hi = s * ns, (s + 1) * ns
        if lo >= na:
            break
        o = res.results[s]["hc_out"]
        hc = np.empty((2, 128, ns), dtype=BF16)
        hc[:, :, 0:nt * T] = (o[:, 0:nt * 2 * T].reshape(128, nt, 2, T)
                              .transpose(2, 0, 1, 3).reshape(2, 128, nt * T))
        if tw:
            hc[:, :, nt * T:] = (o[:, nt * 2 * T:].reshape(128, 2, tw)
                                 .transpose(1, 0, 2))
        n_keep = min(hi, na) - lo
        hv_out[act[lo:lo + n_keep]] = hc[0].T[:n_keep].astype(np.float32)
        cv_out[act[lo:lo + n_keep]] = hc[1].T[:n_keep].astype(np.float32)
    return (hv_out, cv_out), res


def kernel(**inputs):
    out, _ = run(inputs, trace=False)
    return out


# revision 52
# speedup vs baseline: 1.1973x; 1.0047x over previous
"""Trainium2 Bass kernel for nn_NodeRNN (masked single-step LSTM over N nodes).

Strategy: the reference computes the LSTM step everywhere and then keeps the
old state for inactive nodes (ts_mask != 1). Equivalently: gather the active
rows, run the LSTM step on just those, scatter back. The gather/scatter is
pure data routing, done host-side during staging (where the baseline already
transposes); only active nodes ever touch the device. That halves HBM traffic
and every engine's work.

All per-node data is staged FEATURE-MAJOR and in bf16 (tolerance is 2e-2;
bf16 keeps us ~3 orders of magnitude under it), packed into ONE interleaved
dram stream per core laid out [128, NT, 6, T] so each tile is a single DMA of
128 x 12KB contiguous descriptors:
    chunk 0..1: hvv.T      chunk 2..3: Hv.T      chunk 4: hv.T   chunk 5: cv.T

Per 1024-node tile (two 512-node matmul subtiles per PSUM bank):
  x_ps  = [W_pos @ xv.T + b_pos ; W_hid @ X.T + b_hid]   (PE; biases folded
          into the matmul via a ones-row in the aux stream)
  x     = relu(x_ps)                                     (DVE max, -> bf16)
  gates = W_ih @ x + W_hh @ hv.T (+ fused bias via ACT)  (PE)
  i,f,o = sigmoid, g = tanh                              (ACT, -> bf16)
  c_new = f*cv + i*g ; h_new = o*tanh(c_new)             (DVE + one ACT tanh)
Outputs leave as bf16 [128, NT, 2, T]; host scatters them back into f32
copies of hv_tm1/cv_tm1 (inactive rows therefore stay bit-exact).
Emission is software-pipelined (stage A of tile t+1 before stage B of tile t).
"""
import sys

sys.path.insert(0, "/opt/trn_rl_repo")

import numpy as np
import ml_dtypes

import concourse.bacc as bacc
import concourse.tile as tile
from concourse import mybir
from concourse.bass_utils import run_bass_kernel_spmd

f32 = mybir.dt.float32
bf16 = mybir.dt.bfloat16
AF = mybir.ActivationFunctionType
ALU = mybir.AluOpType
BF16 = ml_dtypes.bfloat16

N = 262144
NCORES = 8
T = 1024                  # nodes per tile (DMA + elementwise granularity)
TS = 512                  # matmul subtile (PSUM bank = 512 f32)
EMBED = 64
EDGE_H = 256
NODE_H = 128

# weight block layout: [128, CWF] bf16, free-dim offsets
CO_WHID = 0               # 4 chunks x 128 cols; cols 64:128 of chunk c = W_hid.T chunk
CO_WIH = 512              # W_ih.T [128, 512]
CO_WHH = 1024             # W_hh.T [128, 512]
CO_WP = 1536              # [3, 128]: rows 0:2 = W_pos.T | 0, row 2 = concat(b_pos, b_hid)
CWF = 1664

GATE_FUNCS = [AF.Sigmoid, AF.Sigmoid, AF.Tanh, AF.Sigmoid]  # i, f, g, o

_cached = {}


def build_nc(nt, tw):
    """nt full tiles of T nodes plus one tail tile of tw nodes (multiple of
    128, 0..T-128; the tail is processed FIRST so its small din fill lets the
    PE start early)."""
    ns = nt * T + tw
    nc = bacc.Bacc(target_bir_lowering=False)
    din_d = nc.dram_tensor("din", [128, ns * 6], bf16, kind="ExternalInput")
    aux_d = nc.dram_tensor("aux", [3, ns], bf16, kind="ExternalInput")
    cw_d = nc.dram_tensor("cw", [128, CWF], bf16, kind="ExternalInput")
    cb_d = nc.dram_tensor("cb", [128, 4], f32, kind="ExternalInput")
    out_d = nc.dram_tensor("hc_out", [128, ns * 2], bf16, kind="ExternalOutput")

    def tile_view(dram, t, c):
        w = T if t < nt else tw
        off = t * c * T
        return dram[:, off:off + c * w].rearrange("p (c n) -> p c n", c=c)

    with tile.TileContext(nc) as tc:
        with (
            tc.tile_pool(name="const", bufs=1) as cpool,
            tc.tile_pool(name="din", bufs=6) as dinp,
            tc.tile_pool(name="xsb", bufs=3) as xsbp,
            tc.tile_pool(name="gact", bufs=3) as gactp,
            tc.tile_pool(name="tmp", bufs=3) as tmpp,
            tc.tile_pool(name="hcn", bufs=3) as hcnp,
            tc.tile_pool(name="ps_x", bufs=1, space="PSUM") as psx,
            tc.tile_pool(name="ps_g", bufs=3, space="PSUM") as psg,
        ):
            cw = cpool.tile([128, CWF], bf16)
            nc.sync.dma_start(cw[:], cw_d[:])
            cb = cpool.tile([128, 4], f32)
            nc.sync.dma_start(cb[:], cb_d[:])
            # whole-run xv/ones stream: one small DMA (dispatched after the
            # first tile's din so the first matmuls aren't queued behind it)
            aux_sb = cpool.tile([3, ns], bf16)

            # warmup stream: absorbs the cw DMA wait and accumulates the ~3us
            # of continuous PE activity that ramps the HAM clock to full speed
            # before the first real matmul
            warm = psx.tile([128, T], f32, tag="x")
            for _ in range(16):
                nc.tensor.matmul(warm[:, 0:256], cw[0:3, CO_WP:CO_WP + 128],
                                 cw[0:3, 0:256], start=True, stop=True)

            stash = {}

            def stage_a(t, first=False):
                w = T if t < nt else tw
                src = tile_view(din_d, t, 6)
                din_t = dinp.tile([128, 6, T], bf16, tag="din")
                nc.sync.dma_start(din_t[:, :, 0:w], src[:])
                if first:
                    nc.sync.dma_start(aux_sb[:], aux_d[:])

                # x_ps: partitions 0:64 e_v, 64:128 a_v, biases included via
                # the aux ones-row
                x_ps = psx.tile([128, T], f32, tag="x")
                for k0 in range(0, w, TS):
                    ksl = slice(k0, min(k0 + TS, w))
                    for c in range(4):
                        nc.tensor.matmul(
                            x_ps[:, ksl],
                            cw[:, CO_WHID + 128 * c:CO_WHID + 128 * (c + 1)],
                            din_t[:, c, ksl], start=(c == 0), stop=False)
                    nc.tensor.matmul(
                        x_ps[:, ksl], cw[0:3, CO_WP:CO_WP + 128],
                        aux_sb[0:3, t * T + ksl.start:t * T + ksl.stop],
                        start=False, stop=True)

                # x = relu(x_ps) on the DVE, rounded to bf16 for the gate matmuls
                x_sb = xsbp.tile([128, T], bf16, tag="x_sb")
                nc.vector.tensor_scalar_max(x_sb[:, 0:w], x_ps[:, 0:w], 0.0)
                stash[t] = (din_t, x_sb)

            stash2 = {}

            def stage_b1(t, prev=None):
                """Gate matmuls + gate activations + c_new. tanh(c)/h/out of
                the PREVIOUS tile are emitted between this tile's gate
                activations (stage_b2) so the ACT engine never waits on the
                DVE's c of the tile it is currently finishing."""
                din_t, x_sb = stash.pop(t)
                w = T if t < nt else tw
                hcn = hcnp.tile([128, 2, T], bf16, tag="hcn")
                gact = [None] * 4
                # per gate chunk j: g_ps_j = W_hh.T_j @ hv.T + W_ih.T_j @ x.
                # f first so t1 = f*cv can start early on the DVE.
                for j in (1, 0, 2, 3):
                    if j == 2 and prev is not None:
                        stage_b2(prev)
                    gp = psg.tile([128, T], f32, tag="g")
                    for k0 in range(0, w, TS):
                        ksl = slice(k0, min(k0 + TS, w))
                        nc.tensor.matmul(
                            gp[:, ksl],
                            cw[:, CO_WHH + 128 * j:CO_WHH + 128 * (j + 1)],
                            din_t[:, 4, ksl], start=True, stop=False)
                        nc.tensor.matmul(
                            gp[:, ksl],
                            cw[:, CO_WIH + 128 * j:CO_WIH + 128 * (j + 1)],
                            x_sb[:, ksl], start=False, stop=True)
                    ga = gactp.tile([128, T], bf16, tag=f"g{j}")
                    gact[j] = ga
                    nc.scalar.activation(ga[:, 0:w], gp[:, 0:w], GATE_FUNCS[j],
                                         bias=cb[:, j:j + 1])
                    if j == 1:
                        # t1 = f * cv while the other gates are in flight
                        t1 = tmpp.tile([128, T], bf16, tag="t1")
                        nc.vector.tensor_tensor(t1[:, 0:w], ga[:, 0:w],
                                                din_t[:, 5, 0:w], ALU.mult)
                i_s, f_s, g_t, o_s = gact

                t2 = tmpp.tile([128, T], bf16, tag="t2")
                nc.vector.tensor_tensor(t2[:, 0:w], i_s[:, 0:w], g_t[:, 0:w],
                                        ALU.mult)
                nc.vector.tensor_tensor(hcn[:, 1, 0:w], t1[:, 0:w], t2[:, 0:w],
                                        ALU.add)
                stash2[t] = (hcn, o_s)

            def stage_b2(t, final=False):
                hcn, o_s = stash2.pop(t)
                w = T if t < nt else tw
                dst = tile_view(out_d, t, 2)
                # final tile drains in halves so the very last serial
                # tanh/h/out chain is half-length
                nsp = 2 if final and w > TS else 1
                for h in range(nsp):
                    hsl = slice(h * w // nsp, (h + 1) * w // nsp)
                    th = tmpp.tile([128, T], bf16, tag="th")
                    nc.scalar.activation(th[:, hsl], hcn[:, 1, hsl], AF.Tanh)
                    nc.vector.tensor_tensor(hcn[:, 0, hsl], o_s[:, hsl],
                                            th[:, hsl], ALU.mult)
                    # out goes via the GPSIMD software DGE so the sync HW
                    # queue carries only the din stream; the final tile's out
                    # goes via sync (no din left there) to skip the SWDGE drain
                    eng = nc.sync if final else nc.gpsimd
                    eng.dma_start(dst[:, :, hsl], hcn[:, :, hsl])

            # tail tile first: its half-size din lands soonest, so the PE
            # starts real work earlier; full tiles stream behind it
            order = ([nt] if tw else []) + list(range(nt))
            for i in range(len(order) + 1):
                if i < len(order):
                    stage_a(order[i], first=(i == 0))
                if i >= 1:
                    stage_b1(order[i - 1],
                             prev=order[i - 2] if i >= 2 else None)
            stage_b2(order[-1], final=True)

    nc.finalize()
    return nc


def _stage_weights(W_pos, b_pos, W_hid, b_hid, W_ih, b_ih, W_hh, b_hh):
    cw = np.zeros((128, CWF), dtype=np.float32)
    whid_t = np.ascontiguousarray(W_hid.T)          # [512, 64]
    for c in range(4):
        cw[:, CO_WHID + 128 * c + 64:CO_WHID + 128 * (c + 1)] = whid_t[128 * c:128 * (c + 1)]
    cw[:, CO_WIH:CO_WIH + 512] = W_ih.T             # [128, 512]
    cw[:, CO_WHH:CO_WHH + 512] = W_hh.T
    cw[0:2, CO_WP:CO_WP + 64] = W_pos.T             # [2, 64]
    cw[2, CO_WP:CO_WP + 64] = b_pos
    cw[2, CO_WP + 64:CO_WP + 128] = b_hid
    cb = np.zeros((128, 4), dtype=np.float32)
    cb[:, :] = (b_ih + b_hh).reshape(4, 128).T
    return cw.astype(BF16), cb


def _stage_inputs(Hv_t, hvv_t, xv_t, hv_tm1, cv_tm1, act, ncap, weights):
    """Gather active rows, pad to ncap total, stage feature-major bf16."""
    cw, cb = _stage_weights(**weights)
    actp = np.pad(act, (0, ncap - act.size), mode="edge") if act.size < ncap else act
    ns = ncap // NCORES
    nt, tw = ns // T, ns % T
    in_maps = []
    for s in range(NCORES):
        idx = actp[s * ns:(s + 1) * ns]
        buf = np.empty((ns, 768), dtype=np.float32)
        buf[:, 0:256] = hvv_t[idx]
        buf[:, 256:512] = Hv_t[idx]
        buf[:, 512:640] = hv_tm1[idx]
        buf[:, 640:768] = cv_tm1[idx]
        # per tile: din[p, c, n] = buf_tile[n, c*128+p]
        bufb = buf.astype(BF16)
        din = np.empty((128, ns * 6), dtype=BF16)
        din[:, 0:nt * 6 * T] = (bufb[0:nt * T].reshape(nt, T, 6, 128)
                                .transpose(3, 0, 2, 1).reshape(128, nt * 6 * T))
        if tw:
            din[:, nt * 6 * T:] = (bufb[nt * T:].reshape(tw, 6, 128)
                                   .transpose(2, 1, 0).reshape(128, 6 * tw))
        aux = np.empty((3, ns), dtype=np.float32)
        aux[0:2] = xv_t[idx].T
        aux[2] = 1.0
        in_maps.append(dict(din=din, aux=aux.astype(BF16), cw=cw, cb=cb))
    return in_maps, actp


def run(inputs, trace=False):
    """Stage, run on 8 cores, unstage. Returns ((hv_t, cv_t), BassKernelResults)."""
    inputs = {k: np.asarray(v) for k, v in inputs.items()}
    weights = {k: inputs[k] for k in ["W_pos", "b_pos", "W_hid", "b_hid",
                                      "W_ih", "b_ih", "W_hh", "b_hh"]}
    act = np.flatnonzero(inputs["ts_mask"][:, 0] == 1)
    if act.size == 0:
        return (inputs["hv_tm1"].copy(), inputs["cv_tm1"].copy()), None
    grain = NCORES * 128
    ncap = -(-act.size // grain) * grain
    ns = ncap // NCORES
    nt, tw = ns // T, ns % T

    in_maps, actp = _stage_inputs(inputs["Hv_t"], inputs["hvv_t"], inputs["xv_t"],
                                  inputs["hv_tm1"], inputs["cv_tm1"], act, ncap,
                                  weights)
    if (nt, tw) not in _cached:
        _cached[(nt, tw)] = build_nc(nt, tw)
    res = run_bass_kernel_spmd(_cached[(nt, tw)], in_maps,
                               core_ids=list(range(NCORES)), trace=trace)
    hv_out = inputs["hv_tm1"].astype(np.float32, copy=True)
    cv_out = inputs["cv_tm1"].astype(np.float32, copy=True)
    na = act.size
    for s in range(NCORES):
        lo, hi = s * ns, (s + 1) * ns
        if lo >= na:
            break
        o = res.results[s]["hc_out"]
        hc = np.empty((2, 128, ns), dtype=BF16)
        hc[:, :, 0:nt * T] = (o[:, 0:nt * 2 * T].reshape(128, nt, 2, T)
                              .transpose(2, 0, 1, 3).reshape(2, 128, nt * T))
        if tw:
            hc[:, :, nt * T:] = (o[:, nt * 2 * T:].reshape(128, 2, tw)
                                 .transpose(1, 0, 2))
        n_keep = min(hi, na) - lo
        hv_out[act[lo:lo + n_keep]] = hc[0].T[:n_keep].astype(np.float32)
        cv_out[act[lo:lo + n_keep]] = hc[1].T[:n_keep].astype(np.float32)
    return (hv_out, cv_out), res


def kernel(**inputs):
    out, _ = run(inputs, trace=False)
    return out
